# revision 1
# baseline (speedup 1.0000x reference)
"""GraphSAGE (2-layer, mean aggregation) on 8 Trainium2 NeuronCores.

Strategy:
  - Nodes are sharded contiguously across the 8 cores by destination row.
  - Aggregation (segment-mean over 800k edges) is done as: dma_gather of
    source-node features onto partitions (128 edges/chunk), a host-built
    inv_deg-scaled one-hot DMA'd in as the moving matmul operand, and a
    TensorEngine matmul-accumulate into PSUM per 128-dst block.
  - Hidden states are exchanged between layers with an AllGather
    collective (bf16, row-major) so layer-2 can gather any source row.
  - int16 gather indices can't address 50000 rows, so each block's edges
    are split into lo (src < 32768) and hi streams gathered from two
    slices of the feature table.
"""

import math
from contextlib import ExitStack

import numpy as np
import ml_dtypes

import concourse.bass as bass
import concourse.bacc as bacc
import concourse.mybir as mybir
import concourse.tile as tile
from concourse import bass_utils

P = 128
N_NODES = 50000
N_EDGES = 800000
D_IN = 128
D_HID = 128
D_OUT = 40
N_CORES = 8
LO_SPLIT = 32768          # int16 gather index limit boundary
GRP = 16                  # chunks per dma_gather call

BF16 = ml_dtypes.bfloat16


def _wrap_idxs(idx_flat):
    """dma_gather index layout: idx i lives at [i % 16, i // 16] of a
    16-partition tile, replicated to 128 partitions."""
    n = idx_flat.shape[0]
    assert n % 16 == 0
    w = idx_flat.reshape(n // 16, 16).T.astype(np.int16)  # [16, n/16]
    return np.tile(w, (8, 1))                             # [128, n/16]


def preprocess(edge_index, n_nodes=N_NODES, n_cores=N_CORES, lo_split=LO_SPLIT):
    """Sort/partition edges; build per-core gather indices + one-hot metadata.

    Returns (meta, per_core) where per_core[k] holds the numpy arrays the
    device kernel consumes and meta holds the (uniform) structure sizes.
    """
    src = np.asarray(edge_index[0], dtype=np.int64)
    dst = np.asarray(edge_index[1], dtype=np.int64)
    counts = np.bincount(dst, minlength=n_nodes)
    inv_deg = (1.0 / np.maximum(counts, 1)).astype(np.float32)

    rows_per = n_nodes // n_cores
    nblk = math.ceil(rows_per / P)

    order = np.argsort(dst, kind="stable")
    s_s, d_s = src[order], dst[order]

    # boundaries of each (core, block) segment in the dst-sorted edge list
    blk_edges = {}
    n_lo_max, n_hi_max = 0, 0
    for k in range(n_cores):
        base = k * rows_per
        for b in range(nblk):
            r0 = base + b * P
            r1 = min(base + rows_per, r0 + P)
            e0 = np.searchsorted(d_s, r0, side="left")
            e1 = np.searchsorted(d_s, r1, side="left")
            s_seg, d_seg = s_s[e0:e1], d_s[e0:e1]
            lo_m = s_seg < lo_split
            blk_edges[(k, b)] = (s_seg, d_seg, lo_m, r0)
            n_lo_max = max(n_lo_max, int(lo_m.sum()))
            n_hi_max = max(n_hi_max, int((~lo_m).sum()))

    Llo = max(1, math.ceil(n_lo_max / P))
    Lhi = max(1, math.ceil(n_hi_max / P))
    C_lo, C_hi = nblk * Llo, nblk * Lhi

    per_core = []
    for k in range(n_cores):
        idx_lo = np.zeros((C_lo, P), np.int16)
        idx_hi = np.zeros((C_hi, P), np.int16)
        dst_lo = np.full((C_lo, P), -1.0, np.float32)
        dst_hi = np.full((C_hi, P), -1.0, np.float32)
        val_lo = np.zeros((C_lo, P), np.float32)
        val_hi = np.zeros((C_hi, P), np.float32)
        for b in range(nblk):
            s_seg, d_seg, lo_m, r0 = blk_edges[(k, b)]
            for (sel, idx_a, dst_a, val_a, L, off) in (
                (lo_m, idx_lo, dst_lo, val_lo, Llo, 0),
                (~lo_m, idx_hi, dst_hi, val_hi, Lhi, lo_split),
            ):
                ss = s_seg[sel] - off
                dd = d_seg[sel] - r0
                n = ss.shape[0]
                c0 = b * L
                fl_i = idx_a[c0 : c0 + L].reshape(-1)
                fl_d = dst_a[c0 : c0 + L].reshape(-1)
                fl_v = val_a[c0 : c0 + L].reshape(-1)
                fl_i[:n] = ss.astype(np.int16)
                fl_d[:n] = dd.astype(np.float32)
                fl_v[:n] = inv_deg[d_seg[sel]]
        def onehot(dst_a, val_a, C):
            # [128 partitions(edge), C*128] bf16: chunk c cols [c*128, c*128+128),
            # O[p, c*128 + dst] = val for valid edges
            o = np.zeros((C, P, P), BF16)
            cc, pp = np.nonzero(dst_a >= 0)
            o[cc, pp, dst_a[cc, pp].astype(np.int64)] = val_a[cc, pp].astype(BF16)
            return np.ascontiguousarray(o.transpose(1, 0, 2).reshape(P, C * P))

        per_core.append(
            dict(
                idx_lo=_wrap_idxs(idx_lo.reshape(-1)),
                idx_hi=_wrap_idxs(idx_hi.reshape(-1)),
                o_lo=onehot(dst_lo, val_lo, C_lo),
                o_hi=onehot(dst_hi, val_hi, C_hi),
            )
        )

    meta = dict(
        n_nodes=n_nodes, n_cores=n_cores, rows_per=rows_per, nblk=nblk,
        Llo=Llo, Lhi=Lhi, C_lo=C_lo, C_hi=C_hi, lo_split=lo_split,
    )
    return meta, per_core


def build_graph(nc, m, d_in=D_IN, d_out=D_OUT, debug_skip=()):
    dt = mybir.dt
    alu = mybir.AluOpType
    act = mybir.ActivationFunctionType
    n_nodes, rows_per, nblk = m["n_nodes"], m["rows_per"], m["nblk"]
    Llo, Lhi, C_lo, C_hi = m["Llo"], m["Lhi"], m["C_lo"], m["C_hi"]
    lo_split = m["lo_split"]

    x_all = nc.dram_tensor("x_all", [n_nodes, d_in], dt.bfloat16, kind="ExternalInput")
    xT_d = nc.dram_tensor("xT", [P, rows_per], dt.bfloat16, kind="ExternalInput")
    idx_lo_d = nc.dram_tensor("idx_lo", [P, C_lo * 8], dt.int16, kind="ExternalInput")
    idx_hi_d = nc.dram_tensor("idx_hi", [P, C_hi * 8], dt.int16, kind="ExternalInput")
    o_lo_d = nc.dram_tensor("o_lo", [P, C_lo * P], dt.bfloat16, kind="ExternalInput")
    o_hi_d = nc.dram_tensor("o_hi", [P, C_hi * P], dt.bfloat16, kind="ExternalInput")
    w1l_d = nc.dram_tensor("w1lT", [P, d_in], dt.bfloat16, kind="ExternalInput")
    w1r_d = nc.dram_tensor("w1rT", [P, d_in], dt.bfloat16, kind="ExternalInput")
    w2l_d = nc.dram_tensor("w2lT", [P, d_out], dt.bfloat16, kind="ExternalInput")
    w2r_d = nc.dram_tensor("w2rT", [P, d_out], dt.bfloat16, kind="ExternalInput")
    b1_d = nc.dram_tensor("b1r", [1, d_in], dt.bfloat16, kind="ExternalInput")
    b2_d = nc.dram_tensor("b2r", [1, d_out], dt.bfloat16, kind="ExternalInput")
    out_d = nc.dram_tensor("out", [rows_per, d_out], dt.float32, kind="ExternalOutput")

    with tile.TileContext(nc) as tc, ExitStack() as ctx:
        sb = ctx.enter_context(tc.tile_pool(name="sb", bufs=1))
        dram = ctx.enter_context(tc.tile_pool(name="dram", bufs=1, space="DRAM"))
        psum = ctx.enter_context(tc.tile_pool(name="psum", bufs=8, space="PSUM"))
        glo_p = ctx.enter_context(tc.tile_pool(name="glo", bufs=2))
        ghi_p = ctx.enter_context(tc.tile_pool(name="ghi", bufs=2))
        o_p = ctx.enter_context(tc.tile_pool(name="oh", bufs=4))
        st_p = ctx.enter_context(tc.tile_pool(name="st", bufs=2))

        def load(shape, dtype, src, name):
            t = sb.tile(shape, dtype, name=name)
            nc.sync.dma_start(t[:], src[:])
            return t

        xT_sb = load([P, rows_per], dt.bfloat16, xT_d.ap(), "xT_sb")
        idxlo_sb = load([P, C_lo * 8], dt.int16, idx_lo_d.ap(), "idxlo_sb")
        idxhi_sb = load([P, C_hi * 8], dt.int16, idx_hi_d.ap(), "idxhi_sb")
        w1l_sb = load([P, d_in], dt.bfloat16, w1l_d.ap(), "w1l_sb")
        w1r_sb = load([P, d_in], dt.bfloat16, w1r_d.ap(), "w1r_sb")
        w2l_sb = load([P, d_out], dt.bfloat16, w2l_d.ap(), "w2l_sb")
        w2r_sb = load([P, d_out], dt.bfloat16, w2r_d.ap(), "w2r_sb")
        b1_sb = load([1, d_in], dt.bfloat16, b1_d.ap(), "b1_sb")
        b2_sb = load([1, d_out], dt.bfloat16, b2_d.ap(), "b2_sb")

        ones_sb = sb.tile([1, 512], dt.bfloat16, name="ones_sb")
        nc.vector.memset(ones_sb[:], 1.0)

        meanT = sb.tile([P, rows_per], dt.bfloat16, name="meanT")
        meanhT = sb.tile([P, rows_per], dt.bfloat16, name="meanhT")
        hT = sb.tile([P, rows_per], dt.bfloat16, name="hT")

        hsh = dram.tile([rows_per, d_in], dt.bfloat16, name="hsh")
        hfull = dram.tile([n_nodes, d_in], dt.bfloat16, name="hfull")

        def aggregate(src_ap, outT):
            """outT[:, i] = sum_e 1/deg(i) * src[srcnode(e), :] over edges into i."""
            streams = {
                "lo": dict(C=C_lo, idx=idxlo_sb, ap=src_ap[0:lo_split, :],
                           pool=glo_p, tag="glo", o=o_lo_d),
                "hi": dict(C=C_hi, idx=idxhi_sb, ap=src_ap[lo_split:n_nodes, :],
                           pool=ghi_p, tag="ghi", o=o_hi_d),
            }
            tiles = {}
            qctr = [0]

            def ensure_group(stream, g):
                if (stream, g) in tiles:
                    return tiles[(stream, g)]
                s = streams[stream]
                c0, c1 = g * GRP, min(s["C"], (g + 1) * GRP)
                n = (c1 - c0) * P
                t = s["pool"].tile([P, GRP, P], dt.bfloat16, tag=s["tag"],
                                   name=f"g_{s['tag']}")
                if "gather" in debug_skip:
                    nc.vector.memset(t[:, : c1 - c0, :], 0.0)
                else:
                    nc.gpsimd.dma_gather(
                        t[:, : c1 - c0, :], s["ap"],
                        s["idx"][:, c0 * 8 : c1 * 8],
                        n, n, d_in, elem_step=d_in, single_packet=False,
                        queue_num=qctr[0] % nc.num_swdge_queues,
                    )
                    qctr[0] += 1
                ot = o_p.tile([P, GRP, P], dt.bfloat16, tag="ohv", name="ohv")
                nc.sync.dma_start(ot[:, : c1 - c0, :],
                                  s["o"].ap()[:, c0 * P : c1 * P])
                tiles[(stream, g)] = (t, ot)
                return tiles[(stream, g)]

            for b in range(nblk):
                bs = min(P, rows_per - b * P)
                ps = psum.tile([P, 512], dt.float32, tag="ps", name="ps_agg")
                ops = [("lo", c) for c in range(b * Llo, (b + 1) * Llo)]
                ops += [("hi", c) for c in range(b * Lhi, (b + 1) * Lhi)]
                for i, (stream, c) in enumerate(ops):
                    gt, ot = ensure_group(stream, c // GRP)
                    nc.tensor.matmul(
                        ps[:, :P], lhsT=gt[:, c % GRP, :], rhs=ot[:, c % GRP, :],
                        start=(i == 0), stop=(i == len(ops) - 1),
                    )
                nc.vector.tensor_copy(outT[:, b * P : b * P + bs], ps[:, :bs])

        # ---- layer 1 ----
        aggregate(x_all.ap(), meanT)

        for c0 in range(0, rows_per, 512):
            w = min(512, rows_per - c0)
            ps = psum.tile([P, 512], dt.float32, tag="ps", name="ps_d")
            nc.tensor.matmul(ps[:, :w], lhsT=w1l_sb[:], rhs=meanT[:, c0 : c0 + w],
                             start=True, stop=False)
            nc.tensor.matmul(ps[:, :w], lhsT=w1r_sb[:], rhs=xT_sb[:, c0 : c0 + w],
                             start=False, stop=False)
            nc.tensor.matmul(ps[:, :w], lhsT=b1_sb[:], rhs=ones_sb[:, :w],
                             start=False, stop=True)
            nc.scalar.activation(hT[:, c0 : c0 + w], ps[:, :w], act.Relu)

        for b in range(nblk):
            c0 = b * P
            bs = min(P, rows_per - c0)
            ps = psum.tile([P, 512], dt.float32, tag="ps", name="ps_r")
            nc.tensor.matmul(ps[:bs, :d_in], lhsT=meanT[:, c0 : c0 + bs], rhs=w1l_sb[:],
                             start=True, stop=False)
            nc.tensor.matmul(ps[:bs, :d_in], lhsT=xT_sb[:, c0 : c0 + bs], rhs=w1r_sb[:],
                             start=False, stop=False)
            nc.tensor.matmul(ps[:bs, :d_in], lhsT=ones_sb[:, :bs], rhs=b1_sb[:],
                             start=False, stop=True)
            hrow = st_p.tile([P, d_in], dt.bfloat16, tag="st", name="hrow")
            nc.scalar.activation(hrow[:bs, :], ps[:bs, :d_in], act.Relu)
            nc.sync.dma_start(hsh[c0 : c0 + bs, :], hrow[:bs, :])

        if "collective" in debug_skip:
            nc.sync.dma_start(hfull[0:rows_per, :], hsh[:])
        else:
            nc.gpsimd.collective_compute(
                "AllGather", alu.bypass,
                replica_groups=[list(range(m["n_cores"]))],
                ins=[hsh[:].opt()], outs=[hfull[:].opt()],
            )

        # ---- layer 2 ----
        aggregate(hfull, meanhT)

        for b in range(nblk):
            c0 = b * P
            bs = min(P, rows_per - c0)
            ps = psum.tile([P, 512], dt.float32, tag="ps", name="ps_o")
            nc.tensor.matmul(ps[:bs, :d_out], lhsT=meanhT[:, c0 : c0 + bs], rhs=w2l_sb[:],
                             start=True, stop=False)
            nc.tensor.matmul(ps[:bs, :d_out], lhsT=hT[:, c0 : c0 + bs], rhs=w2r_sb[:],
                             start=False, stop=False)
            nc.tensor.matmul(ps[:bs, :d_out], lhsT=ones_sb[:, :bs], rhs=b2_sb[:],
                             start=False, stop=True)
            ot = st_p.tile([P, d_out], dt.float32, tag="ot", name="ot")
            nc.vector.tensor_copy(ot[:bs, :], ps[:bs, :d_out])
            nc.sync.dma_start(out_d.ap()[c0 : c0 + bs, :], ot[:bs, :])

    return nc


def make_in_maps(inputs, meta, per_core):
    x = np.asarray(inputs["x"], np.float32)
    n_cores, rows_per = meta["n_cores"], meta["rows_per"]
    x_bf = x.astype(BF16)
    w1l = np.asarray(inputs["W1l"], np.float32)
    w1r = np.asarray(inputs["W1r"], np.float32)
    w2l = np.asarray(inputs["W2l"], np.float32)
    w2r = np.asarray(inputs["W2r"], np.float32)
    b1 = np.asarray(inputs["b1"], np.float32)
    b2 = np.asarray(inputs["b2"], np.float32)
    in_maps = []
    for k in range(n_cores):
        r0 = k * rows_per
        pc = per_core[k]
        in_maps.append({
            "x_all": x_bf,
            "xT": np.ascontiguousarray(x[r0 : r0 + rows_per].T).astype(BF16),
            "idx_lo": pc["idx_lo"], "idx_hi": pc["idx_hi"],
            "o_lo": pc["o_lo"], "o_hi": pc["o_hi"],
            "w1lT": np.ascontiguousarray(w1l.T).astype(BF16),
            "w1rT": np.ascontiguousarray(w1r.T).astype(BF16),
            "w2lT": np.ascontiguousarray(w2l.T).astype(BF16),
            "w2rT": np.ascontiguousarray(w2r.T).astype(BF16),
            "b1r": b1[None, :].astype(BF16),
            "b2r": b2[None, :].astype(BF16),
        })
    return in_maps


_CACHE = {}


def _compile(meta):
    key = (meta["Llo"], meta["Lhi"], meta["n_nodes"], meta["rows_per"])
    if key not in _CACHE:
        nc = bacc.Bacc("TRN2", target_bir_lowering=False, debug=False,
                       num_devices=meta["n_cores"], num_swdge_queues=4)
        build_graph(nc, meta)
        nc.compile()
        _CACHE[key] = nc
    return _CACHE[key]


def kernel(**inputs):
    edge_index = np.asarray(inputs["edge_index"])
    meta, per_core = preprocess(edge_index)
    nc = _compile(meta)
    in_maps = make_in_maps(inputs, meta, per_core)
    res = bass_utils.run_bass_kernel_spmd(
        nc, in_maps, core_ids=list(range(meta["n_cores"]))
    )
    out = np.concatenate(
        [res.results[k]["out"] for k in range(meta["n_cores"])], axis=0
    )
    return out.astype(np.float32)

